# revision 1
# baseline (speedup 1.0000x reference)
"""Trainium2 Bass kernel for the dense CNN (pad+border-extrapolate, 4 convs,
pixel shuffle). Data parallel: 2 images per core on 8 cores.

Layouts (per image, spatial maps flat with row stride 266, garbage cols at
row tails are computed but never consumed):
- xp: padded+extrapolated image (272 rows x 266) in DRAM scratch, bf16.
- h1r: (128, S1) SBUF bf16: partition [c | 64+c] = conv1 channel c of
  even|odd rows (interleave mod 2); free = y2*266 + x.
- h2r/h3r: (128, S) bf16: partition [32q + c] = channel c of rows == q mod 4;
  free = y4*266 + x.
- h4r: (128, S4) f32: same mod-4 quads, each quad 16 channels + 16 junk.
Convs are tap-accumulated bf16 matmuls into f32 PSUM, col-tiled across PE
column groups (2x for conv1, 4x for conv2/3/4). K-packing uses zero-padded
stacked weight variants so every matmul is a full-K read at rhs base 0.
"""

import numpy as np
import ml_dtypes

import concourse.bass as bass
import concourse.bacc as bacc
import concourse.tile as tile
import concourse.mybir as mybir
from concourse.bass_utils import run_bass_kernel_spmd

F32 = mybir.dt.float32
BF16 = mybir.dt.bfloat16
AF = mybir.ActivationFunctionType
ALU = mybir.AluOpType

W = 266          # flat row stride
XPROWS = 272     # xp rows incl. zero pad rows 266..271
B4 = 16          # output quad-rows (of 4 rows) per band
NBANDS = 4       # B4*4*NBANDS = 256 output rows
B2 = 2 * B4 + 5  # h1r y2-rows per band (37)
NIMG = 2         # images per core
XP = XPROWS * W

S1 = B2 * W                # 9842
S2 = (B4 + 2) * W          # 4788
S3 = (B4 + 1) * W          # 4522
S4 = B4 * W                # 4256
S4C = B4 * 256             # 4096 (conv4 output, no garbage cols)
PAD = 8                    # tile tail pad (halo reads spill a few elements)

# strip layout: per image 6 depth-blocks of 4 edges x 266
# offset(d, e, pos) = d*1064 + e*266 + pos ; e: 0=rowN 1=rowS 2=colW 3=colE
SLEN = 6 * 4 * W  # 6384


def _ap(t, off, dims):
    return bass.AP(tensor=t.tensor, offset=t.offset + off,
                   ap=[list(d) for d in dims])


def build_nc(debug=()):
    nc = bacc.Bacc("TRN2", target_bir_lowering=False)

    xbf = nc.dram_tensor("xbf", [NIMG, 256, 256], BF16, kind="ExternalInput")
    xe = nc.dram_tensor("xe", [NIMG, 4, 256], F32, kind="ExternalInput")
    w1t = nc.dram_tensor("w1t", [25, 64], BF16, kind="ExternalInput")
    w2 = nc.dram_tensor("w2", [128, 12 * 32], BF16, kind="ExternalInput")
    w3 = nc.dram_tensor("w3", [128, 18 * 32], BF16, kind="ExternalInput")
    w4 = nc.dram_tensor("w4", [128, 18 * 16], BF16, kind="ExternalInput")
    b1d = nc.dram_tensor("b1d", [128, 1], F32, kind="ExternalInput")
    b2d = nc.dram_tensor("b2d", [128, 1], F32, kind="ExternalInput")
    b3d = nc.dram_tensor("b3d", [128, 1], F32, kind="ExternalInput")
    b4d = nc.dram_tensor("b4d", [128, 1], F32, kind="ExternalInput")
    y = nc.dram_tensor("y", [NIMG, 1024, 1024], F32, kind="ExternalOutput")
    xp = nc.dram_tensor("xp", [NIMG, XP], BF16,
                        kind="ExternalOutput" if "xp" in debug else "Internal")
    if "h1r" in debug:
        dh1 = nc.dram_tensor("dh1", [128, S1], BF16, kind="ExternalOutput")
    if "h2r" in debug:
        dh2 = nc.dram_tensor("dh2", [128, S2], BF16, kind="ExternalOutput")
    if "h3r" in debug:
        dh3 = nc.dram_tensor("dh3", [128, S3], BF16, kind="ExternalOutput")

    with tile.TileContext(nc) as tc:
        with tc.tile_pool(name="consts", bufs=1) as consts:
            tw1 = consts.tile([25, 64], BF16)
            tw2 = consts.tile([128, 12 * 32], BF16)
            tw3 = consts.tile([128, 18 * 32], BF16)
            tw4 = consts.tile([128, 18 * 16], BF16)
            tb1 = consts.tile([128, 1], F32)
            tb2 = consts.tile([128, 1], F32)
            tb3 = consts.tile([128, 1], F32)
            tb4 = consts.tile([128, 1], F32)
            for dst, src in ((tw1, w1t), (tw2, w2), (tw3, w3), (tw4, w4),
                             (tb1, b1d), (tb2, b2d), (tb3, b3d), (tb4, b4d)):
                nc.sync.dma_start(out=dst, in_=src[:, :])

            # ------------- border extrapolation (f32 strips) -------------
            with tc.tile_pool(name="strips", bufs=1) as spool:
                st = spool.tile([NIMG, SLEN], F32)
                tmp = spool.tile([NIMG, 4 * W], F32)
                msk = spool.tile([NIMG, 4 * W], mybir.dt.uint8)
                ones = spool.tile([NIMG, 4 * W], F32)
                nc.vector.memset(st, 0.5)
                nc.vector.memset(ones, 1.0)

                for k, doff in ((0, 5 * 1064 + 0 + 5), (1, 0 * 1064 + 266 + 5),
                                (2, 5 * 1064 + 532 + 5), (3, 0 * 1064 + 798 + 5)):
                    nc.sync.dma_start(
                        out=_ap(st, doff, [[SLEN, NIMG], [1, 256]]),
                        in_=xe[:, k, :])

                for i in range(5, 0, -1):
                    im = i - 1
                    L = 264 - 2 * i
                    rg = (5 - 2 * i) * 1064 + 266
                    wg = (7 - 2 * i) * 1064 + 266

                    def vin(k):
                        return _ap(st, i * 1064 + i + k,
                                   [[SLEN, NIMG], [rg, 2], [532, 2], [1, L]])
                    tmpa = _ap(tmp, 0, [[4 * W, NIMG], [266, 4], [1, L]])
                    tmpm = _ap(msk, 0, [[4 * W, NIMG], [266, 4], [1, L]])
                    nc.vector.tensor_tensor(out=tmpa, in0=vin(0), in1=vin(1),
                                            op=ALU.add)
                    nc.vector.tensor_tensor(out=tmpa, in0=tmpa, in1=vin(2),
                                            op=ALU.add)
                    nc.vector.tensor_scalar(out=tmpm, in0=tmpa, scalar1=0.9,
                                            scalar2=None, op0=ALU.is_gt)
                    wdst = _ap(st, im * 1064 + i + 1,
                               [[SLEN, NIMG], [wg, 2], [532, 2], [1, L]])
                    nc.vector.tensor_scalar(out=wdst, in0=tmpa,
                                            scalar1=1.0 / 3.0, scalar2=None,
                                            op0=ALU.mult)
                    mview = _ap(msk, 0, [[4 * W, NIMG], [266, 2], [532, 2], [1, L]])
                    oview = _ap(ones, 0, [[4 * W, NIMG], [266, 2], [532, 2], [1, L]])
                    nc.vector.copy_predicated(out=wdst, mask=mview, data=oview)

                    ut = spool.tile([NIMG, 4], F32, tag="ut")

                    def c22(base, grp, pos):
                        return _ap(st, base, [[SLEN, NIMG], [grp, 2], [pos, 2]])
                    utv = _ap(ut, 0, [[4, NIMG], [2, 2], [1, 2]])
                    # u1: (cy,cxp) = ((cyp,cxp) + (cy,cx+2nx))/2
                    nc.vector.tensor_tensor(
                        out=utv,
                        in0=c22(i * 1064 + i, rg, 265 - 2 * i),
                        in1=c22(im * 1064 + i + 1, wg, 263 - 2 * i), op=ALU.add)
                    nc.vector.tensor_scalar(
                        out=c22(im * 1064 + i, wg, 265 - 2 * i), in0=utv,
                        scalar1=0.5, scalar2=None, op0=ALU.mult)
                    nc.vector.tensor_scalar(
                        out=c22(i * 1064 + 532 + im, rg, 267 - 2 * i), in0=utv,
                        scalar1=0.5, scalar2=None, op0=ALU.mult)
                    # u2: (cyp,cx) = ((cyp,cxp) + (cy+2ny,cx))/2
                    nc.vector.tensor_tensor(
                        out=utv,
                        in0=c22(i * 1064 + i, rg, 265 - 2 * i),
                        in1=c22(im * 1064 + 532 + i + 1, wg, 263 - 2 * i),
                        op=ALU.add)
                    nc.vector.tensor_scalar(
                        out=c22(i * 1064 + im, rg, 267 - 2 * i), in0=utv,
                        scalar1=0.5, scalar2=None, op0=ALU.mult)
                    nc.vector.tensor_scalar(
                        out=c22(im * 1064 + 532 + i, wg, 265 - 2 * i), in0=utv,
                        scalar1=0.5, scalar2=None, op0=ALU.mult)
                    # u3: (cy,cx) = ((cy,cxp) + (cyp,cx))/2
                    nc.vector.tensor_tensor(
                        out=utv,
                        in0=c22(im * 1064 + i, wg, 265 - 2 * i),
                        in1=c22(i * 1064 + im, rg, 267 - 2 * i), op=ALU.add)
                    nc.vector.tensor_scalar(
                        out=c22(im * 1064 + im, wg, 267 - 2 * i), in0=utv,
                        scalar1=0.5, scalar2=None, op0=ALU.mult)
                    nc.vector.tensor_scalar(
                        out=c22(im * 1064 + 532 + im, wg, 267 - 2 * i), in0=utv,
                        scalar1=0.5, scalar2=None, op0=ALU.mult)

                # cast to bf16, reordering into contiguous assembly blocks:
                # [rowN d-major 1596][rowS 1596][colW y-major 1596][colE 1596]
                stb = spool.tile([NIMG, SLEN], BF16)
                nc.vector.tensor_copy(
                    out=_ap(stb, 0, [[SLEN, NIMG], [266, 6], [1, 266]]),
                    in_=_ap(st, 0, [[SLEN, NIMG], [1064, 6], [1, 266]]))
                nc.vector.tensor_copy(
                    out=_ap(stb, 1596, [[SLEN, NIMG], [266, 6], [1, 266]]),
                    in_=_ap(st, 266, [[SLEN, NIMG], [1064, 6], [1, 266]]))
                nc.vector.tensor_copy(
                    out=_ap(stb, 2 * 1596, [[SLEN, NIMG], [6, 266], [1, 6]]),
                    in_=_ap(st, 532, [[SLEN, NIMG], [1, 266], [1064, 6]]))
                nc.vector.tensor_copy(
                    out=_ap(stb, 3 * 1596, [[SLEN, NIMG], [6, 266], [1, 6]]),
                    in_=_ap(st, 798, [[SLEN, NIMG], [1, 266], [1064, 6]]))
                zpad = spool.tile([NIMG, 6 * W], BF16)
                nc.vector.memset(zpad, 0.0)
                # col strips first, then rows (rows authoritative in corners)
                for g in range(NIMG):
                    xo = g * XP
                    so = g * SLEN
                    nc.sync.dma_start(
                        out=_ap(xp[:, :], xo + 0, [[W, 266], [1, 6]]),
                        in_=_ap(stb, so + 2 * 1596, [[SLEN, 1], [1, 1596]]))
                    nc.sync.dma_start(
                        out=_ap(xp[:, :], xo + 260, [[W, 266], [1, 6]]),
                        in_=_ap(stb, so + 3 * 1596, [[SLEN, 1], [1, 1596]]))
                    nc.sync.dma_start(
                        out=_ap(xp[:, :], xo + 0, [[1, 1596]]),
                        in_=_ap(stb, so + 0, [[SLEN, 1], [1, 1596]]))
                    nc.sync.dma_start(
                        out=_ap(xp[:, :], xo + 260 * W, [[1, 1596]]),
                        in_=_ap(stb, so + 1596, [[SLEN, 1], [1, 1596]]))
                    nc.sync.dma_start(
                        out=_ap(xp[:, :], xo + 5 * W + 5,
                                [[W, 256], [1, 256]]),
                        in_=xbf[g, :, :])
                    nc.sync.dma_start(
                        out=_ap(xp[:, :], xo + 266 * W, [[1, 6 * W]]),
                        in_=zpad[g:g + 1, :])

            # --------------- conv pipeline ---------------
            with tc.tile_pool(name="t2col", bufs=1) as tpool, \
                 tc.tile_pool(name="hmaps", bufs=1) as hpool, \
                 tc.tile_pool(name="h4", bufs=2) as h4pool, \
                 tc.tile_pool(name="ps", bufs=8, space="PSUM") as pspool:

                for img in range(NIMG):
                    for band in range(NBANDS):
                        gq = band * B4
                        xrow0 = 4 * gq

                        tev = tpool.tile([25, S1], BF16, tag="tev")
                        tod = tpool.tile([25, S1], BF16, tag="tod")
                        for ky in range(5):
                            nc.sync.dma_start(
                                out=_ap(tev, ky * 5 * S1, [[S1, 5], [1, S1]]),
                                in_=_ap(xp[:, :], img * XP + (xrow0 + ky) * W,
                                        [[1, 5], [2 * W, B2], [1, W]]))
                            nc.sync.dma_start(
                                out=_ap(tod, ky * 5 * S1, [[S1, 5], [1, S1]]),
                                in_=_ap(xp[:, :],
                                        img * XP + (xrow0 + 1 + ky) * W,
                                        [[1, 5], [2 * W, B2], [1, W]]))

                        h1r = hpool.tile([128, S1 + PAD], BF16, tag="h1r")
                        h2r = hpool.tile([128, S2 + PAD], BF16, tag="h2r")
                        h3r = hpool.tile([128, S3 + PAD], BF16, tag="h3r")
                        h4c = h4pool.tile([128, S4C + 16], F32, tag="h4c")

                        # ---- conv1: 1->64, K=25, two col-tiles even/odd ----
                        flip = 0
                        for j in range(0, S1, 512):
                            n = min(512, S1 - j)
                            ps = pspool.tile([128, 512], F32, tag="ps")
                            nc.tensor.matmul(ps[0:64, 0:n], tw1[:, :],
                                             tev[:, j:j + n], start=True,
                                             stop=True, tile_position=(0, 0))
                            nc.tensor.matmul(ps[64:128, 0:n], tw1[:, :],
                                             tod[:, j:j + n], start=True,
                                             stop=True, tile_position=(0, 64))
                            if flip % 2 == 0:
                                nc.scalar.activation(
                                    out=h1r[:, j:j + n], in_=ps[:, 0:n],
                                    func=AF.Relu, bias=tb1[:, :], scale=1.0)
                            else:
                                nc.vector.tensor_scalar(
                                    out=h1r[:, j:j + n], in0=ps[:, 0:n],
                                    scalar1=tb1[:, :], scalar2=0.0,
                                    op0=ALU.add, op1=ALU.max)
                            flip += 1

                        # ---- conv2: 64->32, 3-quad-row groups, 4 col-tiles --
                        # psum A: x in [0,170) stride 170; B: [170,266) stride 96
                        for y4l in range(0, B4 + 2, 3):
                            psa = pspool.tile([128, 512], F32, tag="ps")
                            psb = pspool.tile([128, 512], F32, tag="ps")
                            for c in range(4):
                                first = True
                                for kx in range(3):
                                    for dd in (0, 1):
                                        v = (c % 2) * 2 + dd
                                        lhs = tw2[:, (kx * 4 + v) * 32:
                                                  (kx * 4 + v) * 32 + 32]
                                        roff = ((2 * y4l + (c // 2) + dd) * W
                                                + kx)
                                        last = (kx == 2 and dd == 1)
                                        nc.tensor.matmul(
                                            _ap(psa, 32 * c * 512,
                                                [[512, 32], [170, 3], [1, 170]]),
                                            lhs,
                                            _ap(h1r, roff,
                                                [[S1 + PAD, 128], [532, 3],
                                                 [1, 170]]),
                                            start=first, stop=False,
                                            tile_position=(0, 32 * c),
                                            skip_group_check=True)
                                        nc.tensor.matmul(
                                            _ap(psb, 32 * c * 512,
                                                [[512, 32], [96, 3], [1, 96]]),
                                            lhs,
                                            _ap(h1r, roff + 170,
                                                [[S1 + PAD, 128], [532, 3],
                                                 [1, 96]]),
                                            start=first, stop=last,
                                            tile_position=(0, 32 * c),
                                            skip_group_check=True)
                                        first = False
                            dst0 = _ap(h2r, y4l * W,
                                       [[S2 + PAD, 128], [W, 3], [1, 170]])
                            dst1 = _ap(h2r, y4l * W + 170,
                                       [[S2 + PAD, 128], [W, 3], [1, 96]])
                            pina = _ap(psa, 0, [[512, 128], [170, 3], [1, 170]])
                            pinb = _ap(psb, 0, [[512, 128], [96, 3], [1, 96]])
                            nc.scalar.activation(out=dst0, in_=pina, func=AF.Relu,
                                                 bias=tb2[:, :], scale=1.0)
                            nc.vector.tensor_scalar(out=dst1, in0=pinb,
                                                    scalar1=tb2[:, :],
                                                    scalar2=0.0, op0=ALU.add,
                                                    op1=ALU.max)

                        # ---- conv3/conv4: 4-int K-packed, 4 col-tiles ----
                        def conv34(hin, hinS, hout, wt, bias_t, Sout, M, func):
                            CLSV = {0: ((0, 0),), 1: ((1, 0),),
                                    2: ((2, 0), (3, 1)), 3: ((4, 0), (5, 1))}
                            fl = 0
                            for j in range(0, Sout, 512):
                                n = min(512, Sout - j)
                                ps = pspool.tile([128, 512], F32, tag="ps")
                                for c in range(4):
                                    mms = [(kx, v, dd) for kx in range(3)
                                           for (v, dd) in CLSV[c]]
                                    for idx, (kx, v, dd) in enumerate(mms):
                                        lhs = wt[:, (kx * 6 + v) * M:
                                                 (kx * 6 + v) * M + M]
                                        ro = j + dd * W + kx
                                        nc.tensor.matmul(
                                            _ap(ps, 32 * c * 512,
                                                [[512, M], [1, n]]),
                                            lhs,
                                            _ap(hin, ro,
                                                [[hinS + PAD, 128], [1, n]]),
                                            start=(idx == 0),
                                            stop=(idx == len(mms) - 1),
                                            tile_position=(0, 32 * c),
                                            skip_group_check=True)
                                if func is AF.Sigmoid:
                                    nc.scalar.activation(
                                        out=hout[:, j:j + n], in_=ps[:, 0:n],
                                        func=func, bias=bias_t[:, :], scale=1.0)
                                elif fl % 2 == 0:
                                    nc.scalar.activation(
                                        out=hout[:, j:j + n], in_=ps[:, 0:n],
                                        func=AF.Relu, bias=bias_t[:, :],
                                        scale=1.0)
                                else:
                                    nc.vector.tensor_scalar(
                                        out=hout[:, j:j + n], in0=ps[:, 0:n],
                                        scalar1=bias_t[:, :], scalar2=0.0,
                                        op0=ALU.add, op1=ALU.max)
                                fl += 1

                        conv34(h2r, S2, h3r, tw3, tb3, S3, 32, AF.Relu)

                        # ---- conv4: 3-quad-row groups, 256-stride output ----
                        CLSV = {0: ((0, 0),), 1: ((1, 0),),
                                2: ((2, 0), (3, 1)), 3: ((4, 0), (5, 1))}
                        for y4l in range(0, B4, 3):
                            nr = min(3, B4 - y4l)
                            psa = pspool.tile([128, 512], F32, tag="ps")
                            psb = pspool.tile([128, 512], F32, tag="ps")
                            for c in range(4):
                                mms = [(kx, v, dd) for kx in range(3)
                                       for (v, dd) in CLSV[c]]
                                for idx, (kx, v, dd) in enumerate(mms):
                                    lhs = tw4[:, (kx * 6 + v) * 16:
                                              (kx * 6 + v) * 16 + 16]
                                    ro = y4l * W + dd * W + kx
                                    nc.tensor.matmul(
                                        _ap(psa, 32 * c * 512,
                                            [[512, 16], [170, nr], [1, 170]]),
                                        lhs,
                                        _ap(h3r, ro,
                                            [[S3 + PAD, 128], [W, nr],
                                             [1, 170]]),
                                        start=(idx == 0), stop=False,
                                        tile_position=(0, 32 * c),
                                        skip_group_check=True)
                                    nc.tensor.matmul(
                                        _ap(psb, 32 * c * 512,
                                            [[512, 16], [96, nr], [1, 96]]),
                                        lhs,
                                        _ap(h3r, ro + 170,
                                            [[S3 + PAD, 128], [W, nr],
                                             [1, 96]]),
                                        start=(idx == 0),
                                        stop=(idx == len(mms) - 1),
                                        tile_position=(0, 32 * c),
                                        skip_group_check=True)
                            # evac B first (spills 10 cols into next row head),
                            # then A overwrites the spill. sigmoid+bias on ACT.
                            nc.scalar.activation(
                                out=_ap(h4c, y4l * 256 + 170,
                                        [[S4C + 16, 128], [256, nr], [1, 96]]),
                                in_=_ap(psb, 0, [[512, 128], [96, nr], [1, 96]]),
                                func=AF.Sigmoid, bias=tb4[:, :], scale=1.0)
                            nc.scalar.activation(
                                out=_ap(h4c, y4l * 256,
                                        [[S4C + 16, 128], [256, nr], [1, 170]]),
                                in_=_ap(psa, 0, [[512, 128], [170, nr], [1, 170]]),
                                func=AF.Sigmoid, bias=tb4[:, :], scale=1.0)

                        # ---- pixel shuffle DMA (split per quad, i, j) ----
                        # y[img, 4*(4*y4+q)+i, 4*x+j] = h4c[32q+4i+j, y4*256+x]
                        for q in range(4):
                            for i in range(4):
                                for jj in range(4):
                                    nc.sync.dma_start(
                                        out=_ap(y[:, :, :],
                                                img * 1024 * 1024
                                                + (16 * gq + 4 * q + i) * 1024
                                                + jj,
                                                [[16 * 1024, B4], [4, 256]]),
                                        in_=_ap(h4c,
                                                (32 * q + 4 * i + jj)
                                                * (S4C + 16),
                                                [[S4C + 16, 1], [1, S4C]]))

                        if "h1r" in debug and img == 0 and band == 0:
                            nc.sync.dma_start(out=dh1[:, :],
                                              in_=h1r[:, 0:S1])
                        if "h2r" in debug and img == 0 and band == 0:
                            nc.sync.dma_start(out=dh2[:, :],
                                              in_=h2r[:, 0:S2])
                        if "h3r" in debug and img == 0 and band == 0:
                            nc.sync.dma_start(out=dh3[:, :],
                                              in_=h3r[:, 0:S3])

    nc.finalize()
    return nc


def host_inputs(x, W1, b1, W2, b2, W3, b3, W4, b4, core):
    """Build the per-core input map (images 2*core, 2*core+1)."""
    xi = np.asarray(x[2 * core:2 * core + 2], dtype=np.float32)
    bf = ml_dtypes.bfloat16

    xe = np.stack([xi[:, 0, :], xi[:, 255, :], xi[:, :, 0], xi[:, :, 255]],
                  axis=1).astype(np.float32)

    w1t = np.ascontiguousarray(np.asarray(W1)[:, 0].reshape(64, 25).T)

    w2v = np.zeros((128, 12 * 32), np.float32)
    for kx in range(3):
        Wk = [np.asarray(W2)[:, :, ky, kx].T for ky in range(3)]  # (64,32)
        Z = np.zeros_like(Wk[0])
        var = [np.concatenate([Wk[0], Wk[1]], 0),
               np.concatenate([Wk[2], Z], 0),
               np.concatenate([Z, Wk[0]], 0),
               np.concatenate([Wk[1], Wk[2]], 0)]
        for v in range(4):
            w2v[:, (kx * 4 + v) * 32:(kx * 4 + v) * 32 + 32] = var[v]

    def conv34_vars(Wc, M):
        w = np.zeros((128, 18 * M), np.float32)
        for kx in range(3):
            Wk = [np.asarray(Wc)[:, :, ky, kx].T for ky in range(3)]  # (32,M)
            Z = np.zeros_like(Wk[0])
            var = [np.concatenate([Wk[0], Wk[1], Wk[2], Z], 0),
                   np.concatenate([Z, Wk[0], Wk[1], Wk[2]], 0),
                   np.concatenate([Z, Z, Wk[0], Wk[1]], 0),
                   np.concatenate([Wk[2], Z, Z, Z], 0),
                   np.concatenate([Z, Z, Z, Wk[0]], 0),
                   np.concatenate([Wk[1], Wk[2], Z, Z], 0)]
            for v in range(6):
                w[:, (kx * 6 + v) * M:(kx * 6 + v) * M + M] = var[v]
        return w

    w3v = conv34_vars(W3, 32)
    w4v = conv34_vars(W4, 16)

    b1x = np.concatenate([b1, b1]).reshape(128, 1).astype(np.float32)
    b2x = np.tile(b2, 4).reshape(128, 1).astype(np.float32)
    b3x = np.tile(b3, 4).reshape(128, 1).astype(np.float32)
    b4x = np.zeros((128, 1), np.float32)
    for q in range(4):
        b4x[32 * q:32 * q + 16, 0] = b4

    return {
        "xbf": xi.astype(bf),
        "xe": xe,
        "w1t": w1t.astype(bf),
        "w2": w2v.astype(bf),
        "w3": w3v.astype(bf),
        "w4": w4v.astype(bf),
        "b1d": b1x, "b2d": b2x, "b3d": b3x, "b4d": b4x,
    }


_NC_CACHE = {}


def _get_nc(debug=()):
    key = tuple(sorted(debug))
    if key not in _NC_CACHE:
        _NC_CACHE[key] = build_nc(debug)
    return _NC_CACHE[key]


def kernel(x, W1, b1, W2, b2, W3, b3, W4, b4, _debug=(), _results=None):
    nc = _get_nc(_debug)
    in_maps = [host_inputs(x, W1, b1, W2, b2, W3, b3, W4, b4, core)
               for core in range(8)]
    res = run_bass_kernel_spmd(nc, in_maps, core_ids=list(range(8)))
    if _results is not None:
        _results.extend(res.results)
    out = np.concatenate([r["y"] for r in res.results], axis=0)
    return np.ascontiguousarray(out.astype(np.float32))



# revision 4
# speedup vs baseline: 16.5047x; 16.5047x over previous
"""Trainium2 Bass kernel for the dense CNN (pad+border-extrapolate, 4 convs,
pixel shuffle). Data parallel: 2 images per core on 8 cores.

Layouts (per image, spatial maps flat with row stride 266, garbage cols at
row tails are computed but never consumed):
- xp: padded+extrapolated image (272 rows x 266) in DRAM scratch, bf16.
- h1r: (128, S1) SBUF bf16: partition [c | 64+c] = conv1 channel c of
  even|odd rows (interleave mod 2); free = y2*266 + x.
- h2r/h3r: (128, S) bf16: partition [32q + c] = channel c of rows == q mod 4;
  free = y4*266 + x.
- h4r: (128, S4) f32: same mod-4 quads, each quad 16 channels + 16 junk.
Convs are tap-accumulated bf16 matmuls into f32 PSUM, col-tiled across PE
column groups (2x for conv1, 4x for conv2/3/4). K-packing uses zero-padded
stacked weight variants so every matmul is a full-K read at rhs base 0.
"""

import numpy as np
import ml_dtypes

import concourse.bass as bass
import concourse.bacc as bacc
import concourse.tile as tile
import concourse.mybir as mybir
from concourse.bass_utils import run_bass_kernel_spmd

F32 = mybir.dt.float32
BF16 = mybir.dt.bfloat16
AF = mybir.ActivationFunctionType
ALU = mybir.AluOpType

W = 266          # flat row stride
XPROWS = 272     # xp rows incl. zero pad rows 266..271
B4 = 16          # output quad-rows (of 4 rows) per band
NBANDS = 4       # B4*4*NBANDS = 256 output rows
B2 = 2 * B4 + 5  # h1r y2-rows per band (37)
NIMG = 2         # images per core
XP = XPROWS * W

S1 = B2 * W                # 9842
S2 = (B4 + 2) * W          # 4788
S3 = (B4 + 1) * W          # 4522
S4 = B4 * W                # 4256
S4C = B4 * 256             # 4096 (conv4 output, no garbage cols)
PAD = 8                    # tile tail pad (halo reads spill a few elements)

# strip layout: per image 6 depth-blocks of 4 edges x 266
# offset(d, e, pos) = d*1064 + e*266 + pos ; e: 0=rowN 1=rowS 2=colW 3=colE
SLEN = 6 * 4 * W  # 6384


def _ap(t, off, dims):
    return bass.AP(tensor=t.tensor, offset=t.offset + off,
                   ap=[list(d) for d in dims])


def build_nc(debug=()):
    nc = bacc.Bacc("TRN2", target_bir_lowering=False)

    xbf = nc.dram_tensor("xbf", [NIMG, 256, 256], BF16, kind="ExternalInput")
    xe = nc.dram_tensor("xe", [NIMG, 4, 256], F32, kind="ExternalInput")
    w1t = nc.dram_tensor("w1t", [25, 64], BF16, kind="ExternalInput")
    w2 = nc.dram_tensor("w2", [128, 12 * 32], BF16, kind="ExternalInput")
    w3 = nc.dram_tensor("w3", [128, 18 * 32], BF16, kind="ExternalInput")
    w4 = nc.dram_tensor("w4", [128, 18 * 16], BF16, kind="ExternalInput")
    b1d = nc.dram_tensor("b1d", [128, 1], F32, kind="ExternalInput")
    b2d = nc.dram_tensor("b2d", [128, 1], F32, kind="ExternalInput")
    b3d = nc.dram_tensor("b3d", [128, 1], F32, kind="ExternalInput")
    b4d = nc.dram_tensor("b4d", [128, 1], F32, kind="ExternalInput")
    y = nc.dram_tensor("y", [NIMG, 1024, 1024], F32, kind="ExternalOutput")
    hs = nc.dram_tensor("hs", [NIMG, NBANDS, 256, 1024], F32, kind="Internal")
    xp = nc.dram_tensor("xp", [NIMG, XP], BF16,
                        kind="ExternalOutput" if "xp" in debug else "Internal")
    if "h1r" in debug:
        dh1 = nc.dram_tensor("dh1", [128, S1], BF16, kind="ExternalOutput")
    if "h2r" in debug:
        dh2 = nc.dram_tensor("dh2", [128, S2], BF16, kind="ExternalOutput")
    if "h3r" in debug:
        dh3 = nc.dram_tensor("dh3", [128, S3], BF16, kind="ExternalOutput")

    with tile.TileContext(nc) as tc:
        with tc.tile_pool(name="consts", bufs=1) as consts:
            tw1 = consts.tile([25, 64], BF16)
            tw2 = consts.tile([128, 12 * 32], BF16)
            tw3 = consts.tile([128, 18 * 32], BF16)
            tw4 = consts.tile([128, 18 * 16], BF16)
            tb1 = consts.tile([128, 1], F32)
            tb2 = consts.tile([128, 1], F32)
            tb3 = consts.tile([128, 1], F32)
            tb4 = consts.tile([128, 1], F32)
            for dst, src in ((tw1, w1t), (tw2, w2), (tw3, w3), (tw4, w4),
                             (tb1, b1d), (tb2, b2d), (tb3, b3d), (tb4, b4d)):
                nc.sync.dma_start(out=dst, in_=src[:, :])

            # ------------- border extrapolation (f32 strips) -------------
            with tc.tile_pool(name="strips", bufs=1) as spool:
                st = spool.tile([NIMG, SLEN], F32)
                tmp = spool.tile([NIMG, 4 * W], F32)
                msk = spool.tile([NIMG, 4 * W], mybir.dt.uint8)
                ones = spool.tile([NIMG, 4 * W], F32)
                nc.vector.memset(st, 0.5)
                nc.vector.memset(ones, 1.0)

                for k, doff in ((0, 5 * 1064 + 0 + 5), (1, 0 * 1064 + 266 + 5),
                                (2, 5 * 1064 + 532 + 5), (3, 0 * 1064 + 798 + 5)):
                    nc.sync.dma_start(
                        out=_ap(st, doff, [[SLEN, NIMG], [1, 256]]),
                        in_=xe[:, k, :])

                for i in range(5, 0, -1):
                    im = i - 1
                    L = 264 - 2 * i
                    rg = (5 - 2 * i) * 1064 + 266
                    wg = (7 - 2 * i) * 1064 + 266

                    def vin(k):
                        return _ap(st, i * 1064 + i + k,
                                   [[SLEN, NIMG], [rg, 2], [532, 2], [1, L]])
                    tmpa = _ap(tmp, 0, [[4 * W, NIMG], [266, 4], [1, L]])
                    tmpm = _ap(msk, 0, [[4 * W, NIMG], [266, 4], [1, L]])
                    nc.vector.tensor_tensor(out=tmpa, in0=vin(0), in1=vin(1),
                                            op=ALU.add)
                    nc.vector.tensor_tensor(out=tmpa, in0=tmpa, in1=vin(2),
                                            op=ALU.add)
                    nc.vector.tensor_scalar(out=tmpm, in0=tmpa, scalar1=0.9,
                                            scalar2=None, op0=ALU.is_gt)
                    wdst = _ap(st, im * 1064 + i + 1,
                               [[SLEN, NIMG], [wg, 2], [532, 2], [1, L]])
                    nc.vector.tensor_scalar(out=wdst, in0=tmpa,
                                            scalar1=1.0 / 3.0, scalar2=None,
                                            op0=ALU.mult)
                    mview = _ap(msk, 0, [[4 * W, NIMG], [266, 2], [532, 2], [1, L]])
                    oview = _ap(ones, 0, [[4 * W, NIMG], [266, 2], [532, 2], [1, L]])
                    nc.vector.copy_predicated(out=wdst, mask=mview, data=oview)

                    ut = spool.tile([NIMG, 4], F32, tag="ut")

                    def c22(base, grp, pos):
                        return _ap(st, base, [[SLEN, NIMG], [grp, 2], [pos, 2]])
                    utv = _ap(ut, 0, [[4, NIMG], [2, 2], [1, 2]])
                    # u1: (cy,cxp) = ((cyp,cxp) + (cy,cx+2nx))/2
                    nc.vector.tensor_tensor(
                        out=utv,
                        in0=c22(i * 1064 + i, rg, 265 - 2 * i),
                        in1=c22(im * 1064 + i + 1, wg, 263 - 2 * i), op=ALU.add)
                    nc.vector.tensor_scalar(
                        out=c22(im * 1064 + i, wg, 265 - 2 * i), in0=utv,
                        scalar1=0.5, scalar2=None, op0=ALU.mult)
                    nc.vector.tensor_scalar(
                        out=c22(i * 1064 + 532 + im, rg, 267 - 2 * i), in0=utv,
                        scalar1=0.5, scalar2=None, op0=ALU.mult)
                    # u2: (cyp,cx) = ((cyp,cxp) + (cy+2ny,cx))/2
                    nc.vector.tensor_tensor(
                        out=utv,
                        in0=c22(i * 1064 + i, rg, 265 - 2 * i),
                        in1=c22(im * 1064 + 532 + i + 1, wg, 263 - 2 * i),
                        op=ALU.add)
                    nc.vector.tensor_scalar(
                        out=c22(i * 1064 + im, rg, 267 - 2 * i), in0=utv,
                        scalar1=0.5, scalar2=None, op0=ALU.mult)
                    nc.vector.tensor_scalar(
                        out=c22(im * 1064 + 532 + i, wg, 265 - 2 * i), in0=utv,
                        scalar1=0.5, scalar2=None, op0=ALU.mult)
                    # u3: (cy,cx) = ((cy,cxp) + (cyp,cx))/2
                    nc.vector.tensor_tensor(
                        out=utv,
                        in0=c22(im * 1064 + i, wg, 265 - 2 * i),
                        in1=c22(i * 1064 + im, rg, 267 - 2 * i), op=ALU.add)
                    nc.vector.tensor_scalar(
                        out=c22(im * 1064 + im, wg, 267 - 2 * i), in0=utv,
                        scalar1=0.5, scalar2=None, op0=ALU.mult)
                    nc.vector.tensor_scalar(
                        out=c22(im * 1064 + 532 + im, wg, 267 - 2 * i), in0=utv,
                        scalar1=0.5, scalar2=None, op0=ALU.mult)

                # cast to bf16, reordering into contiguous assembly blocks:
                # [rowN d-major 1596][rowS 1596][colW y-major 1596][colE 1596]
                stb = spool.tile([NIMG, SLEN], BF16)
                nc.vector.tensor_copy(
                    out=_ap(stb, 0, [[SLEN, NIMG], [266, 6], [1, 266]]),
                    in_=_ap(st, 0, [[SLEN, NIMG], [1064, 6], [1, 266]]))
                nc.vector.tensor_copy(
                    out=_ap(stb, 1596, [[SLEN, NIMG], [266, 6], [1, 266]]),
                    in_=_ap(st, 266, [[SLEN, NIMG], [1064, 6], [1, 266]]))
                nc.vector.tensor_copy(
                    out=_ap(stb, 2 * 1596, [[SLEN, NIMG], [6, 266], [1, 6]]),
                    in_=_ap(st, 532, [[SLEN, NIMG], [1, 266], [1064, 6]]))
                nc.vector.tensor_copy(
                    out=_ap(stb, 3 * 1596, [[SLEN, NIMG], [6, 266], [1, 6]]),
                    in_=_ap(st, 798, [[SLEN, NIMG], [1, 266], [1064, 6]]))
                zpad = spool.tile([NIMG, 6 * W], BF16)
                nc.vector.memset(zpad, 0.0)
                # col strips first, then rows (rows authoritative in corners)
                for g in range(NIMG):
                    xo = g * XP
                    so = g * SLEN
                    nc.sync.dma_start(
                        out=_ap(xp[:, :], xo + 0, [[W, 266], [1, 6]]),
                        in_=_ap(stb, so + 2 * 1596, [[SLEN, 1], [1, 1596]]))
                    nc.sync.dma_start(
                        out=_ap(xp[:, :], xo + 260, [[W, 266], [1, 6]]),
                        in_=_ap(stb, so + 3 * 1596, [[SLEN, 1], [1, 1596]]))
                    nc.sync.dma_start(
                        out=_ap(xp[:, :], xo + 0, [[1, 1596]]),
                        in_=_ap(stb, so + 0, [[SLEN, 1], [1, 1596]]))
                    nc.sync.dma_start(
                        out=_ap(xp[:, :], xo + 260 * W, [[1, 1596]]),
                        in_=_ap(stb, so + 1596, [[SLEN, 1], [1, 1596]]))
                    nc.sync.dma_start(
                        out=_ap(xp[:, :], xo + 5 * W + 5,
                                [[W, 256], [1, 256]]),
                        in_=xbf[g, :, :])
                    nc.sync.dma_start(
                        out=_ap(xp[:, :], xo + 266 * W, [[1, 6 * W]]),
                        in_=zpad[g:g + 1, :])

            # --------------- conv pipeline ---------------
            with tc.tile_pool(name="t2col", bufs=1) as tpool, \
                 tc.tile_pool(name="hmaps", bufs=1) as hpool, \
                 tc.tile_pool(name="h4", bufs=2) as h4pool, \
                 tc.tile_pool(name="ps", bufs=8, space="PSUM") as pspool:

                for img in range(NIMG):
                    for band in range(NBANDS):
                        gq = band * B4
                        xrow0 = 4 * gq

                        tev = tpool.tile([25, S1], BF16, tag="tev")
                        tod = tpool.tile([25, S1], BF16, tag="tod")
                        for ky in range(5):
                            nc.sync.dma_start(
                                out=_ap(tev, ky * 5 * S1, [[S1, 5], [1, S1]]),
                                in_=_ap(xp[:, :], img * XP + (xrow0 + ky) * W,
                                        [[1, 5], [2 * W, B2], [1, W]]))
                            nc.sync.dma_start(
                                out=_ap(tod, ky * 5 * S1, [[S1, 5], [1, S1]]),
                                in_=_ap(xp[:, :],
                                        img * XP + (xrow0 + 1 + ky) * W,
                                        [[1, 5], [2 * W, B2], [1, W]]))

                        h1r = hpool.tile([128, S1 + PAD], BF16, tag="h1r")
                        h2r = hpool.tile([128, S2 + PAD], BF16, tag="h2r")
                        h3r = hpool.tile([128, S3 + PAD], BF16, tag="h3r")
                        h4c = h4pool.tile([128, S4C + 16], F32, tag="h4c")

                        # ---- conv1: 1->64, K=25, two col-tiles even/odd ----
                        flip = 0
                        for j in range(0, S1, 512):
                            n = min(512, S1 - j)
                            ps = pspool.tile([128, 512], F32, tag="ps")
                            nc.tensor.matmul(ps[0:64, 0:n], tw1[:, :],
                                             tev[:, j:j + n], start=True,
                                             stop=True, tile_position=(0, 0))
                            nc.tensor.matmul(ps[64:128, 0:n], tw1[:, :],
                                             tod[:, j:j + n], start=True,
                                             stop=True, tile_position=(0, 64))
                            if flip % 2 == 0:
                                nc.scalar.activation(
                                    out=h1r[:, j:j + n], in_=ps[:, 0:n],
                                    func=AF.Relu, bias=tb1[:, :], scale=1.0)
                            else:
                                nc.vector.tensor_scalar(
                                    out=h1r[:, j:j + n], in0=ps[:, 0:n],
                                    scalar1=tb1[:, :], scalar2=0.0,
                                    op0=ALU.add, op1=ALU.max)
                            flip += 1

                        # ---- conv2: 64->32, 3-quad-row groups, 4 col-tiles --
                        # psum A: x in [0,170) stride 170; B: [170,266) stride 96
                        for y4l in range(0, B4 + 2, 3):
                            psa = pspool.tile([128, 512], F32, tag="ps")
                            psb = pspool.tile([128, 512], F32, tag="ps")
                            for c in range(4):
                                first = True
                                for kx in range(3):
                                    for dd in (0, 1):
                                        v = (c % 2) * 2 + dd
                                        lhs = tw2[:, (kx * 4 + v) * 32:
                                                  (kx * 4 + v) * 32 + 32]
                                        roff = ((2 * y4l + (c // 2) + dd) * W
                                                + kx)
                                        last = (kx == 2 and dd == 1)
                                        nc.tensor.matmul(
                                            _ap(psa, 32 * c * 512,
                                                [[512, 32], [170, 3], [1, 170]]),
                                            lhs,
                                            _ap(h1r, roff,
                                                [[S1 + PAD, 128], [532, 3],
                                                 [1, 170]]),
                                            start=first, stop=False,
                                            tile_position=(0, 32 * c),
                                            skip_group_check=True)
                                        nc.tensor.matmul(
                                            _ap(psb, 32 * c * 512,
                                                [[512, 32], [96, 3], [1, 96]]),
                                            lhs,
                                            _ap(h1r, roff + 170,
                                                [[S1 + PAD, 128], [532, 3],
                                                 [1, 96]]),
                                            start=first, stop=last,
                                            tile_position=(0, 32 * c),
                                            skip_group_check=True)
                                        first = False
                            dst0 = _ap(h2r, y4l * W,
                                       [[S2 + PAD, 128], [W, 3], [1, 170]])
                            dst1 = _ap(h2r, y4l * W + 170,
                                       [[S2 + PAD, 128], [W, 3], [1, 96]])
                            pina = _ap(psa, 0, [[512, 128], [170, 3], [1, 170]])
                            pinb = _ap(psb, 0, [[512, 128], [96, 3], [1, 96]])
                            nc.scalar.activation(out=dst0, in_=pina, func=AF.Relu,
                                                 bias=tb2[:, :], scale=1.0)
                            nc.vector.tensor_scalar(out=dst1, in0=pinb,
                                                    scalar1=tb2[:, :],
                                                    scalar2=0.0, op0=ALU.add,
                                                    op1=ALU.max)

                        # ---- conv3/conv4: 4-int K-packed, 4 col-tiles ----
                        def conv34(hin, hinS, hout, wt, bias_t, Sout, M, func):
                            CLSV = {0: ((0, 0),), 1: ((1, 0),),
                                    2: ((2, 0), (3, 1)), 3: ((4, 0), (5, 1))}
                            fl = 0
                            for j in range(0, Sout, 512):
                                n = min(512, Sout - j)
                                ps = pspool.tile([128, 512], F32, tag="ps")
                                for c in range(4):
                                    mms = [(kx, v, dd) for kx in range(3)
                                           for (v, dd) in CLSV[c]]
                                    for idx, (kx, v, dd) in enumerate(mms):
                                        lhs = wt[:, (kx * 6 + v) * M:
                                                 (kx * 6 + v) * M + M]
                                        ro = j + dd * W + kx
                                        nc.tensor.matmul(
                                            _ap(ps, 32 * c * 512,
                                                [[512, M], [1, n]]),
                                            lhs,
                                            _ap(hin, ro,
                                                [[hinS + PAD, 128], [1, n]]),
                                            start=(idx == 0),
                                            stop=(idx == len(mms) - 1),
                                            tile_position=(0, 32 * c),
                                            skip_group_check=True)
                                if func is AF.Sigmoid:
                                    nc.scalar.activation(
                                        out=hout[:, j:j + n], in_=ps[:, 0:n],
                                        func=func, bias=bias_t[:, :], scale=1.0)
                                elif fl % 2 == 0:
                                    nc.scalar.activation(
                                        out=hout[:, j:j + n], in_=ps[:, 0:n],
                                        func=AF.Relu, bias=bias_t[:, :],
                                        scale=1.0)
                                else:
                                    nc.vector.tensor_scalar(
                                        out=hout[:, j:j + n], in0=ps[:, 0:n],
                                        scalar1=bias_t[:, :], scalar2=0.0,
                                        op0=ALU.add, op1=ALU.max)
                                fl += 1

                        conv34(h2r, S2, h3r, tw3, tb3, S3, 32, AF.Relu)

                        # ---- conv4: 3-quad-row groups, 256-stride output ----
                        CLSV = {0: ((0, 0),), 1: ((1, 0),),
                                2: ((2, 0), (3, 1)), 3: ((4, 0), (5, 1))}
                        for y4l in range(0, B4, 3):
                            nr = min(3, B4 - y4l)
                            psa = pspool.tile([128, 512], F32, tag="ps")
                            psb = pspool.tile([128, 512], F32, tag="ps")
                            for c in range(4):
                                mms = [(kx, v, dd) for kx in range(3)
                                       for (v, dd) in CLSV[c]]
                                for idx, (kx, v, dd) in enumerate(mms):
                                    lhs = tw4[:, (kx * 6 + v) * 16:
                                              (kx * 6 + v) * 16 + 16]
                                    ro = y4l * W + dd * W + kx
                                    nc.tensor.matmul(
                                        _ap(psa, 32 * c * 512,
                                            [[512, 16], [170, nr], [1, 170]]),
                                        lhs,
                                        _ap(h3r, ro,
                                            [[S3 + PAD, 128], [W, nr],
                                             [1, 170]]),
                                        start=(idx == 0), stop=False,
                                        tile_position=(0, 32 * c),
                                        skip_group_check=True)
                                    nc.tensor.matmul(
                                        _ap(psb, 32 * c * 512,
                                            [[512, 16], [96, nr], [1, 96]]),
                                        lhs,
                                        _ap(h3r, ro + 170,
                                            [[S3 + PAD, 128], [W, nr],
                                             [1, 96]]),
                                        start=(idx == 0),
                                        stop=(idx == len(mms) - 1),
                                        tile_position=(0, 32 * c),
                                        skip_group_check=True)
                            # evac B first (spills 10 cols into next row head),
                            # then A overwrites the spill. sigmoid+bias on ACT.
                            nc.scalar.activation(
                                out=_ap(h4c, y4l * 256 + 170,
                                        [[S4C + 16, 128], [256, nr], [1, 96]]),
                                in_=_ap(psb, 0, [[512, 128], [96, nr], [1, 96]]),
                                func=AF.Sigmoid, bias=tb4[:, :], scale=1.0)
                            nc.scalar.activation(
                                out=_ap(h4c, y4l * 256,
                                        [[S4C + 16, 128], [256, nr], [1, 170]]),
                                in_=_ap(psa, 0, [[512, 128], [170, nr], [1, 170]]),
                                func=AF.Sigmoid, bias=tb4[:, :], scale=1.0)

                        # ---- pixel shuffle: DRAM bounce + DVE interleave ----
                        # h4c[32q+4i+j, y4*256+x] -> y[256*band+r', 4x+j],
                        # r' = 16*y4 + 4q + i. SBUF DMA partition dims must
                        # step by 1, so the partition permute happens on the
                        # DRAM side: hop1 writes hs in r'-major j-blocked
                        # order; hop2 reloads with partition = r' % 128; DVE
                        # interleaves j in-partition; one contiguous store.
                        hsb = (img * NBANDS + band) * 256 * 1024
                        for q in range(4):
                            for i in range(4):
                                nc.sync.dma_start(
                                    out=_ap(hs[:, :, :, :],
                                            hsb + (4 * q + i) * 1024,
                                            [[256, 4], [16 * 1024, 16],
                                             [1, 256]]),
                                    in_=_ap(h4c, (32 * q + 4 * i)
                                            * (S4C + 16),
                                            [[S4C + 16, 4], [256, 16],
                                             [1, 256]]))
                        op = h4pool.tile([128, 2048], F32, tag="op")
                        ot = h4pool.tile([128, 2048], F32, tag="ot")
                        nc.sync.dma_start(
                            out=_ap(op, 0, [[2048, 128], [1024, 2],
                                            [1, 1024]]),
                            in_=_ap(hs[:, :, :, :], hsb,
                                    [[1024, 128], [128 * 1024, 2],
                                     [1, 1024]]))
                        for half in range(2):
                            for j4 in range(4):
                                nc.vector.tensor_copy(
                                    out=_ap(ot, half * 1024 + j4,
                                            [[2048, 128], [4, 256]]),
                                    in_=_ap(op, half * 1024 + j4 * 256,
                                            [[2048, 128], [1, 256]]))
                        nc.sync.dma_start(
                            out=_ap(y[:, :, :],
                                    img * 1024 * 1024 + 16 * gq * 1024,
                                    [[1024, 128], [128 * 1024, 2],
                                     [1, 1024]]),
                            in_=_ap(ot, 0, [[2048, 128], [1024, 2],
                                            [1, 1024]]))

                        if "h1r" in debug and img == 0 and band == 0:
                            nc.sync.dma_start(out=dh1[:, :],
                                              in_=h1r[:, 0:S1])
                        if "h2r" in debug and img == 0 and band == 0:
                            nc.sync.dma_start(out=dh2[:, :],
                                              in_=h2r[:, 0:S2])
                        if "h3r" in debug and img == 0 and band == 0:
                            nc.sync.dma_start(out=dh3[:, :],
                                              in_=h3r[:, 0:S3])

    nc.finalize()
    return nc


def host_inputs(x, W1, b1, W2, b2, W3, b3, W4, b4, core):
    """Build the per-core input map (images 2*core, 2*core+1)."""
    xi = np.asarray(x[2 * core:2 * core + 2], dtype=np.float32)
    bf = ml_dtypes.bfloat16

    xe = np.stack([xi[:, 0, :], xi[:, 255, :], xi[:, :, 0], xi[:, :, 255]],
                  axis=1).astype(np.float32)

    w1t = np.ascontiguousarray(np.asarray(W1)[:, 0].reshape(64, 25).T)

    w2v = np.zeros((128, 12 * 32), np.float32)
    for kx in range(3):
        Wk = [np.asarray(W2)[:, :, ky, kx].T for ky in range(3)]  # (64,32)
        Z = np.zeros_like(Wk[0])
        var = [np.concatenate([Wk[0], Wk[1]], 0),
               np.concatenate([Wk[2], Z], 0),
               np.concatenate([Z, Wk[0]], 0),
               np.concatenate([Wk[1], Wk[2]], 0)]
        for v in range(4):
            w2v[:, (kx * 4 + v) * 32:(kx * 4 + v) * 32 + 32] = var[v]

    def conv34_vars(Wc, M):
        w = np.zeros((128, 18 * M), np.float32)
        for kx in range(3):
            Wk = [np.asarray(Wc)[:, :, ky, kx].T for ky in range(3)]  # (32,M)
            Z = np.zeros_like(Wk[0])
            var = [np.concatenate([Wk[0], Wk[1], Wk[2], Z], 0),
                   np.concatenate([Z, Wk[0], Wk[1], Wk[2]], 0),
                   np.concatenate([Z, Z, Wk[0], Wk[1]], 0),
                   np.concatenate([Wk[2], Z, Z, Z], 0),
                   np.concatenate([Z, Z, Z, Wk[0]], 0),
                   np.concatenate([Wk[1], Wk[2], Z, Z], 0)]
            for v in range(6):
                w[:, (kx * 6 + v) * M:(kx * 6 + v) * M + M] = var[v]
        return w

    w3v = conv34_vars(W3, 32)
    w4v = conv34_vars(W4, 16)

    b1x = np.concatenate([b1, b1]).reshape(128, 1).astype(np.float32)
    b2x = np.tile(b2, 4).reshape(128, 1).astype(np.float32)
    b3x = np.tile(b3, 4).reshape(128, 1).astype(np.float32)
    b4x = np.zeros((128, 1), np.float32)
    for q in range(4):
        b4x[32 * q:32 * q + 16, 0] = b4

    return {
        "xbf": xi.astype(bf),
        "xe": xe,
        "w1t": w1t.astype(bf),
        "w2": w2v.astype(bf),
        "w3": w3v.astype(bf),
        "w4": w4v.astype(bf),
        "b1d": b1x, "b2d": b2x, "b3d": b3x, "b4d": b4x,
    }


_NC_CACHE = {}


def _get_nc(debug=()):
    key = tuple(sorted(debug))
    if key not in _NC_CACHE:
        _NC_CACHE[key] = build_nc(debug)
    return _NC_CACHE[key]


def kernel(x, W1, b1, W2, b2, W3, b3, W4, b4, _debug=(), _results=None):
    nc = _get_nc(_debug)
    in_maps = [host_inputs(x, W1, b1, W2, b2, W3, b3, W4, b4, core)
               for core in range(8)]
    res = run_bass_kernel_spmd(nc, in_maps, core_ids=list(range(8)))
    if _results is not None:
        _results.extend(res.results)
    out = np.concatenate([r["y"] for r in res.results], axis=0)
    return np.ascontiguousarray(out.astype(np.float32))



# revision 8
# speedup vs baseline: 16.7397x; 1.0142x over previous
"""Trainium2 Bass kernel for the dense CNN (pad+border-extrapolate, 4 convs,
pixel shuffle). Data parallel: 2 images per core on 8 cores.

Layouts (per image, spatial maps flat with row stride 266, garbage cols at
row tails are computed but never consumed):
- xp: padded+extrapolated image (272 rows x 266) in DRAM scratch, bf16.
- h1r: (128, S1) SBUF bf16: partition [c | 64+c] = conv1 channel c of
  even|odd rows (interleave mod 2); free = y2*266 + x.
- h2r/h3r: (128, S) bf16: partition [32q + c] = channel c of rows == q mod 4;
  free = y4*266 + x.
- h4r: (128, S4) f32: same mod-4 quads, each quad 16 channels + 16 junk.
Convs are tap-accumulated bf16 matmuls into f32 PSUM, col-tiled across PE
column groups (2x for conv1, 4x for conv2/3/4). K-packing uses zero-padded
stacked weight variants so every matmul is a full-K read at rhs base 0.
"""

import numpy as np
import ml_dtypes

import concourse.bass as bass
import concourse.bacc as bacc
import concourse.tile as tile
import concourse.mybir as mybir
from concourse.bass_utils import run_bass_kernel_spmd

F32 = mybir.dt.float32
BF16 = mybir.dt.bfloat16
AF = mybir.ActivationFunctionType
ALU = mybir.AluOpType

W = 266          # flat row stride
XPROWS = 272     # xp rows incl. zero pad rows 266..271
B4 = 16          # output quad-rows (of 4 rows) per band
NBANDS = 4       # B4*4*NBANDS = 256 output rows
B2 = 2 * B4 + 5  # h1r y2-rows per band (37)
NIMG = 2         # images per core
XP = XPROWS * W

S1 = B2 * W                # 9842
S2 = (B4 + 2) * W          # 4788
S3 = (B4 + 1) * W          # 4522
S4 = B4 * W                # 4256
S4C = B4 * 256             # 4096 (conv4 output, no garbage cols)
PAD = 8                    # tile tail pad (halo reads spill a few elements)

# strip layout: per image 6 depth-blocks of 4 edges x 266
# offset(d, e, pos) = d*1064 + e*266 + pos ; e: 0=rowN 1=rowS 2=colW 3=colE
SLEN = 6 * 4 * W  # 6384


def _ap(t, off, dims):
    return bass.AP(tensor=t.tensor, offset=t.offset + off,
                   ap=[list(d) for d in dims])


def build_nc(debug=()):
    nc = bacc.Bacc("TRN2", target_bir_lowering=False)

    xbf = nc.dram_tensor("xbf", [NIMG, 256, 256], BF16, kind="ExternalInput")
    xe = nc.dram_tensor("xe", [NIMG, 4, 256], F32, kind="ExternalInput")
    w1t = nc.dram_tensor("w1t", [25, 64], BF16, kind="ExternalInput")
    w2 = nc.dram_tensor("w2", [128, 9 * 128], BF16, kind="ExternalInput")
    w3 = nc.dram_tensor("w3", [128, 6 * 128], BF16, kind="ExternalInput")
    w4 = nc.dram_tensor("w4", [128, 6 * 128], BF16, kind="ExternalInput")
    b1d = nc.dram_tensor("b1d", [128, 1], F32, kind="ExternalInput")
    b2d = nc.dram_tensor("b2d", [128, 1], F32, kind="ExternalInput")
    b3d = nc.dram_tensor("b3d", [128, 1], F32, kind="ExternalInput")
    b4d = nc.dram_tensor("b4d", [128, 1], F32, kind="ExternalInput")
    y = nc.dram_tensor("y", [NIMG, 1024, 1024], F32, kind="ExternalOutput")
    hs = nc.dram_tensor("hs", [NIMG, NBANDS, 256, 1024], F32, kind="Internal")
    xp = nc.dram_tensor("xp", [NIMG, XP], BF16,
                        kind="ExternalOutput" if "xp" in debug else "Internal")
    if "h1r" in debug:
        dh1 = nc.dram_tensor("dh1", [128, S1], BF16, kind="ExternalOutput")
    if "h2r" in debug:
        dh2 = nc.dram_tensor("dh2", [128, S2], BF16, kind="ExternalOutput")
    if "h3r" in debug:
        dh3 = nc.dram_tensor("dh3", [128, S3], BF16, kind="ExternalOutput")

    with tile.TileContext(nc) as tc:
        with tc.tile_pool(name="consts", bufs=1) as consts:
            tw1 = consts.tile([25, 64], BF16)
            tw2 = consts.tile([128, 9 * 128], BF16)
            tw3 = consts.tile([128, 6 * 128], BF16)
            tw4 = consts.tile([128, 6 * 128], BF16)
            tb1 = consts.tile([128, 1], F32)
            tb2 = consts.tile([128, 1], F32)
            tb3 = consts.tile([128, 1], F32)
            tb4 = consts.tile([128, 1], F32)
            for dst, src in ((tw1, w1t), (tw2, w2), (tw3, w3), (tw4, w4),
                             (tb1, b1d), (tb2, b2d), (tb3, b3d), (tb4, b4d)):
                nc.sync.dma_start(out=dst, in_=src[:, :])

            # ------------- border extrapolation (f32 strips) -------------
            with tc.tile_pool(name="strips", bufs=1) as spool:
                st = spool.tile([NIMG, SLEN], F32)
                tmp = spool.tile([NIMG, 4 * W], F32)
                msk = spool.tile([NIMG, 4 * W], mybir.dt.uint8)
                ones = spool.tile([NIMG, 4 * W], F32)
                nc.vector.memset(st, 0.5)
                nc.vector.memset(ones, 1.0)

                for k, doff in ((0, 5 * 1064 + 0 + 5), (1, 0 * 1064 + 266 + 5),
                                (2, 5 * 1064 + 532 + 5), (3, 0 * 1064 + 798 + 5)):
                    nc.sync.dma_start(
                        out=_ap(st, doff, [[SLEN, NIMG], [1, 256]]),
                        in_=xe[:, k, :])

                for i in range(5, 0, -1):
                    im = i - 1
                    L = 264 - 2 * i
                    rg = (5 - 2 * i) * 1064 + 266
                    wg = (7 - 2 * i) * 1064 + 266

                    def vin(k):
                        return _ap(st, i * 1064 + i + k,
                                   [[SLEN, NIMG], [rg, 2], [532, 2], [1, L]])
                    tmpa = _ap(tmp, 0, [[4 * W, NIMG], [266, 4], [1, L]])
                    tmpm = _ap(msk, 0, [[4 * W, NIMG], [266, 4], [1, L]])
                    nc.vector.tensor_tensor(out=tmpa, in0=vin(0), in1=vin(1),
                                            op=ALU.add)
                    nc.vector.tensor_tensor(out=tmpa, in0=tmpa, in1=vin(2),
                                            op=ALU.add)
                    nc.vector.tensor_scalar(out=tmpm, in0=tmpa, scalar1=0.9,
                                            scalar2=None, op0=ALU.is_gt)
                    wdst = _ap(st, im * 1064 + i + 1,
                               [[SLEN, NIMG], [wg, 2], [532, 2], [1, L]])
                    nc.vector.tensor_scalar(out=wdst, in0=tmpa,
                                            scalar1=1.0 / 3.0, scalar2=None,
                                            op0=ALU.mult)
                    mview = _ap(msk, 0, [[4 * W, NIMG], [266, 2], [532, 2], [1, L]])
                    oview = _ap(ones, 0, [[4 * W, NIMG], [266, 2], [532, 2], [1, L]])
                    nc.vector.copy_predicated(out=wdst, mask=mview, data=oview)

                    ut = spool.tile([NIMG, 4], F32, tag="ut")

                    def c22(base, grp, pos):
                        return _ap(st, base, [[SLEN, NIMG], [grp, 2], [pos, 2]])
                    utv = _ap(ut, 0, [[4, NIMG], [2, 2], [1, 2]])
                    # u1: (cy,cxp) = ((cyp,cxp) + (cy,cx+2nx))/2
                    nc.vector.tensor_tensor(
                        out=utv,
                        in0=c22(i * 1064 + i, rg, 265 - 2 * i),
                        in1=c22(im * 1064 + i + 1, wg, 263 - 2 * i), op=ALU.add)
                    nc.vector.tensor_scalar(
                        out=c22(im * 1064 + i, wg, 265 - 2 * i), in0=utv,
                        scalar1=0.5, scalar2=None, op0=ALU.mult)
                    nc.vector.tensor_scalar(
                        out=c22(i * 1064 + 532 + im, rg, 267 - 2 * i), in0=utv,
                        scalar1=0.5, scalar2=None, op0=ALU.mult)
                    # u2: (cyp,cx) = ((cyp,cxp) + (cy+2ny,cx))/2
                    nc.vector.tensor_tensor(
                        out=utv,
                        in0=c22(i * 1064 + i, rg, 265 - 2 * i),
                        in1=c22(im * 1064 + 532 + i + 1, wg, 263 - 2 * i),
                        op=ALU.add)
                    nc.vector.tensor_scalar(
                        out=c22(i * 1064 + im, rg, 267 - 2 * i), in0=utv,
                        scalar1=0.5, scalar2=None, op0=ALU.mult)
                    nc.vector.tensor_scalar(
                        out=c22(im * 1064 + 532 + i, wg, 265 - 2 * i), in0=utv,
                        scalar1=0.5, scalar2=None, op0=ALU.mult)
                    # u3: (cy,cx) = ((cy,cxp) + (cyp,cx))/2
                    nc.vector.tensor_tensor(
                        out=utv,
                        in0=c22(im * 1064 + i, wg, 265 - 2 * i),
                        in1=c22(i * 1064 + im, rg, 267 - 2 * i), op=ALU.add)
                    nc.vector.tensor_scalar(
                        out=c22(im * 1064 + im, wg, 267 - 2 * i), in0=utv,
                        scalar1=0.5, scalar2=None, op0=ALU.mult)
                    nc.vector.tensor_scalar(
                        out=c22(im * 1064 + 532 + im, wg, 267 - 2 * i), in0=utv,
                        scalar1=0.5, scalar2=None, op0=ALU.mult)

                # cast to bf16, reordering into contiguous assembly blocks:
                # [rowN d-major 1596][rowS 1596][colW y-major 1596][colE 1596]
                stb = spool.tile([NIMG, SLEN], BF16)
                nc.vector.tensor_copy(
                    out=_ap(stb, 0, [[SLEN, NIMG], [266, 6], [1, 266]]),
                    in_=_ap(st, 0, [[SLEN, NIMG], [1064, 6], [1, 266]]))
                nc.vector.tensor_copy(
                    out=_ap(stb, 1596, [[SLEN, NIMG], [266, 6], [1, 266]]),
                    in_=_ap(st, 266, [[SLEN, NIMG], [1064, 6], [1, 266]]))
                nc.vector.tensor_copy(
                    out=_ap(stb, 2 * 1596, [[SLEN, NIMG], [6, 266], [1, 6]]),
                    in_=_ap(st, 532, [[SLEN, NIMG], [1, 266], [1064, 6]]))
                nc.vector.tensor_copy(
                    out=_ap(stb, 3 * 1596, [[SLEN, NIMG], [6, 266], [1, 6]]),
                    in_=_ap(st, 798, [[SLEN, NIMG], [1, 266], [1064, 6]]))
                zpad = spool.tile([NIMG, 6 * W], BF16)
                nc.vector.memset(zpad, 0.0)
                # col strips first, then rows (rows authoritative in corners)
                for g in range(NIMG):
                    xo = g * XP
                    so = g * SLEN
                    nc.sync.dma_start(
                        out=_ap(xp[:, :], xo + 0, [[W, 266], [1, 6]]),
                        in_=_ap(stb, so + 2 * 1596, [[SLEN, 1], [1, 1596]]))
                    nc.sync.dma_start(
                        out=_ap(xp[:, :], xo + 260, [[W, 266], [1, 6]]),
                        in_=_ap(stb, so + 3 * 1596, [[SLEN, 1], [1, 1596]]))
                    nc.sync.dma_start(
                        out=_ap(xp[:, :], xo + 0, [[1, 1596]]),
                        in_=_ap(stb, so + 0, [[SLEN, 1], [1, 1596]]))
                    nc.sync.dma_start(
                        out=_ap(xp[:, :], xo + 260 * W, [[1, 1596]]),
                        in_=_ap(stb, so + 1596, [[SLEN, 1], [1, 1596]]))
                    nc.sync.dma_start(
                        out=_ap(xp[:, :], xo + 5 * W + 5,
                                [[W, 256], [1, 256]]),
                        in_=xbf[g, :, :])
                    nc.sync.dma_start(
                        out=_ap(xp[:, :], xo + 266 * W, [[1, 6 * W]]),
                        in_=zpad[g:g + 1, :])

            # --------------- conv pipeline ---------------
            with tc.tile_pool(name="t2col", bufs=1) as tpool, \
                 tc.tile_pool(name="hmaps", bufs=1) as hpool, \
                 tc.tile_pool(name="h4", bufs=2) as h4pool, \
                 tc.tile_pool(name="ps", bufs=8, space="PSUM") as pspool:

                for img in range(NIMG):
                    for band in range(NBANDS):
                        gq = band * B4
                        xrow0 = 4 * gq

                        tev = tpool.tile([25, S1], BF16, tag="tev")
                        tod = tpool.tile([25, S1], BF16, tag="tod")
                        for ky in range(5):
                            nc.sync.dma_start(
                                out=_ap(tev, ky * 5 * S1, [[S1, 5], [1, S1]]),
                                in_=_ap(xp[:, :], img * XP + (xrow0 + ky) * W,
                                        [[1, 5], [2 * W, B2], [1, W]]))
                            nc.sync.dma_start(
                                out=_ap(tod, ky * 5 * S1, [[S1, 5], [1, S1]]),
                                in_=_ap(xp[:, :],
                                        img * XP + (xrow0 + 1 + ky) * W,
                                        [[1, 5], [2 * W, B2], [1, W]]))

                        h1r = hpool.tile([128, S1 + PAD], BF16, tag="h1r")
                        h2r = hpool.tile([128, S2 + PAD], BF16, tag="h2r")
                        h3r = hpool.tile([128, S3 + PAD], BF16, tag="h3r")
                        h4c = h4pool.tile([128, S4C + 16], F32, tag="h4c")

                        # ---- conv1: 1->64, K=25, two col-tiles even/odd ----
                        flip = 0
                        for j in range(0, S1, 512):
                            n = min(512, S1 - j)
                            ps = pspool.tile([128, 512], F32, tag="ps")
                            nc.tensor.matmul(ps[0:64, 0:n], tw1[:, :],
                                             tev[:, j:j + n], start=True,
                                             stop=True, tile_position=(0, 0))
                            nc.tensor.matmul(ps[64:128, 0:n], tw1[:, :],
                                             tod[:, j:j + n], start=True,
                                             stop=True, tile_position=(0, 64))
                            if flip % 2 == 0:
                                nc.scalar.activation(
                                    out=h1r[:, j:j + n], in_=ps[:, 0:n],
                                    func=AF.Relu, bias=tb1[:, :], scale=1.0)
                            else:
                                nc.vector.tensor_scalar(
                                    out=h1r[:, j:j + n], in0=ps[:, 0:n],
                                    scalar1=tb1[:, :], scalar2=0.0,
                                    op0=ALU.add, op1=ALU.max)
                            flip += 1

                        # ---- conv2: 64->32, M=128 = 4 out rows x 32c ----
                        # psum partition 32*hl + c; 9 streams per quad row:
                        # 3 kx x 3 row-pairs p (rhs y2 = 2Y+p), K=128 = 64c'
                        # x even/odd, lhsT variant zero where dy=2p+e-hl
                        # outside [0,2].
                        for Y in range(B4 + 2):
                            ps = pspool.tile([128, 512], F32, tag="ps")
                            idx = 0
                            for kx in range(3):
                                for p in range(3):
                                    nc.tensor.matmul(
                                        ps[:, 0:266],
                                        tw2[:, (kx * 3 + p) * 128:
                                            (kx * 3 + p) * 128 + 128],
                                        _ap(h1r, (2 * Y + p) * W + kx,
                                            [[S1 + PAD, 128], [1, 266]]),
                                        start=(idx == 0), stop=(idx == 8))
                                    idx += 1
                            dst = _ap(h2r, Y * W, [[S2 + PAD, 128], [1, 266]])
                            if Y % 2 == 0:
                                nc.scalar.activation(
                                    out=dst, in_=ps[:, 0:266], func=AF.Relu,
                                    bias=tb2[:, :], scale=1.0)
                            else:
                                nc.vector.tensor_scalar(
                                    out=dst, in0=ps[:, 0:266],
                                    scalar1=tb2[:, :], scalar2=0.0,
                                    op0=ALU.add, op1=ALU.max)

                        # ---- conv3/conv4: M=128 = 4 out rows x 32c ----
                        # 6 streams per quad row: 3 kx x 2 quad variants
                        # (rhs quads Y, Y+1), K=128 = 32c' x 4 rows, lhsT
                        # zero where dy=q+4v-hl outside [0,2].
                        def conv34(hin, hinS, nq, wt, bias_t, emit):
                            for Y in range(nq):
                                ps = pspool.tile([128, 512], F32, tag="ps")
                                idx = 0
                                for kx in range(3):
                                    for v in range(2):
                                        nc.tensor.matmul(
                                            ps[:, 0:266],
                                            wt[:, (kx * 2 + v) * 128:
                                               (kx * 2 + v) * 128 + 128],
                                            _ap(hin, (Y + v) * W + kx,
                                                [[hinS + PAD, 128], [1, 266]]),
                                            start=(idx == 0), stop=(idx == 5))
                                        idx += 1
                                emit(Y, ps)

                        def emit3(Y, ps):
                            dst = _ap(h3r, Y * W, [[S3 + PAD, 128], [1, 266]])
                            if Y % 2 == 0:
                                nc.scalar.activation(
                                    out=dst, in_=ps[:, 0:266], func=AF.Relu,
                                    bias=tb3[:, :], scale=1.0)
                            else:
                                nc.vector.tensor_scalar(
                                    out=dst, in0=ps[:, 0:266],
                                    scalar1=tb3[:, :], scalar2=0.0,
                                    op0=ALU.add, op1=ALU.max)

                        def emit4(Y, ps):
                            nc.scalar.activation(
                                out=_ap(h4c, Y * 256,
                                        [[S4C + 16, 128], [1, 256]]),
                                in_=ps[:, 0:256], func=AF.Sigmoid,
                                bias=tb4[:, :], scale=1.0)

                        conv34(h2r, S2, B4 + 1, tw3, tb3, emit3)
                        conv34(h3r, S3, B4, tw4, tb4, emit4)

                        # ---- pixel shuffle: DRAM bounce + DVE interleave ----
                        # h4c[32q+4i+j, y4*256+x] -> y[256*band+r', 4x+j],
                        # r' = 16*y4 + 4q + i. SBUF DMA partition dims must
                        # step by 1, so the partition permute happens on the
                        # DRAM side: hop1 writes hs in r'-major j-blocked
                        # order; hop2 reloads with partition = r' % 128; DVE
                        # interleaves j in-partition; one contiguous store.
                        hsb = (img * NBANDS + band) * 256 * 1024
                        for q in range(4):
                            for i in range(4):
                                nc.sync.dma_start(
                                    out=_ap(hs[:, :, :, :],
                                            hsb + (4 * q + i) * 1024,
                                            [[256, 4], [16 * 1024, 16],
                                             [1, 256]]),
                                    in_=_ap(h4c, (32 * q + 4 * i)
                                            * (S4C + 16),
                                            [[S4C + 16, 4], [256, 16],
                                             [1, 256]]))
                        op = h4pool.tile([128, 2048], F32, tag="op")
                        ot = h4pool.tile([128, 2048], F32, tag="ot")
                        nc.sync.dma_start(
                            out=_ap(op, 0, [[2048, 128], [1024, 2],
                                            [1, 1024]]),
                            in_=_ap(hs[:, :, :, :], hsb,
                                    [[1024, 128], [128 * 1024, 2],
                                     [1, 1024]]))
                        for half in range(2):
                            for j4 in range(4):
                                nc.vector.tensor_copy(
                                    out=_ap(ot, half * 1024 + j4,
                                            [[2048, 128], [4, 256]]),
                                    in_=_ap(op, half * 1024 + j4 * 256,
                                            [[2048, 128], [1, 256]]))
                        nc.sync.dma_start(
                            out=_ap(y[:, :, :],
                                    img * 1024 * 1024 + 16 * gq * 1024,
                                    [[1024, 128], [128 * 1024, 2],
                                     [1, 1024]]),
                            in_=_ap(ot, 0, [[2048, 128], [1024, 2],
                                            [1, 1024]]))

                        if "h1r" in debug and img == 0 and band == 0:
                            nc.sync.dma_start(out=dh1[:, :],
                                              in_=h1r[:, 0:S1])
                        if "h2r" in debug and img == 0 and band == 0:
                            nc.sync.dma_start(out=dh2[:, :],
                                              in_=h2r[:, 0:S2])
                        if "h3r" in debug and img == 0 and band == 0:
                            nc.sync.dma_start(out=dh3[:, :],
                                              in_=h3r[:, 0:S3])

    nc.finalize()
    return nc


def host_inputs(x, W1, b1, W2, b2, W3, b3, W4, b4, core):
    """Build the per-core input map (images 2*core, 2*core+1)."""
    xi = np.asarray(x[2 * core:2 * core + 2], dtype=np.float32)
    bf = ml_dtypes.bfloat16

    xe = np.stack([xi[:, 0, :], xi[:, 255, :], xi[:, :, 0], xi[:, :, 255]],
                  axis=1).astype(np.float32)

    w1t = np.ascontiguousarray(np.asarray(W1)[:, 0].reshape(64, 25).T)

    # conv2 variants: lhsT[64e+c', 32hl+c] = W2[c, c', dy, kx],
    # dy = 2p + e - hl, zero outside [0, 2]
    W2n = np.asarray(W2)
    w2v = np.zeros((128, 9 * 128), np.float32)
    for kx in range(3):
        for p in range(3):
            blk = np.zeros((128, 128), np.float32)
            for hl in range(4):
                for e in range(2):
                    dy = 2 * p + e - hl
                    if 0 <= dy <= 2:
                        blk[64 * e:64 * e + 64,
                            32 * hl:32 * hl + 32] = W2n[:, :, dy, kx].T
            w2v[:, (kx * 3 + p) * 128:(kx * 3 + p + 1) * 128] = blk

    # conv3/4 variants: lhsT[32q+c', 32hl+c] = W[c, c', dy, kx],
    # dy = q + 4v - hl, zero outside [0, 2]
    def conv34_vars(Wc, M):
        Wn = np.asarray(Wc)
        w = np.zeros((128, 6 * 128), np.float32)
        for kx in range(3):
            for v in range(2):
                blk = np.zeros((128, 128), np.float32)
                for hl in range(4):
                    for q in range(4):
                        dy = q + 4 * v - hl
                        if 0 <= dy <= 2:
                            blk[32 * q:32 * q + 32,
                                32 * hl:32 * hl + M] = Wn[:, :, dy, kx].T
                w[:, (kx * 2 + v) * 128:(kx * 2 + v + 1) * 128] = blk
        return w

    w3v = conv34_vars(W3, 32)
    w4v = conv34_vars(W4, 16)

    b1x = np.concatenate([b1, b1]).reshape(128, 1).astype(np.float32)
    b2x = np.tile(b2, 4).reshape(128, 1).astype(np.float32)
    b3x = np.tile(b3, 4).reshape(128, 1).astype(np.float32)
    b4x = np.zeros((128, 1), np.float32)
    for q in range(4):
        b4x[32 * q:32 * q + 16, 0] = b4

    return {
        "xbf": xi.astype(bf),
        "xe": xe,
        "w1t": w1t.astype(bf),
        "w2": w2v.astype(bf),
        "w3": w3v.astype(bf),
        "w4": w4v.astype(bf),
        "b1d": b1x, "b2d": b2x, "b3d": b3x, "b4d": b4x,
    }


_NC_CACHE = {}


def _get_nc(debug=()):
    key = tuple(sorted(debug))
    if key not in _NC_CACHE:
        _NC_CACHE[key] = build_nc(debug)
    return _NC_CACHE[key]


def kernel(x, W1, b1, W2, b2, W3, b3, W4, b4, _debug=(), _results=None):
    nc = _get_nc(_debug)
    in_maps = [host_inputs(x, W1, b1, W2, b2, W3, b3, W4, b4, core)
               for core in range(8)]
    res = run_bass_kernel_spmd(nc, in_maps, core_ids=list(range(8)))
    if _results is not None:
        _results.extend(res.results)
    out = np.concatenate([r["y"] for r in res.results], axis=0)
    return np.ascontiguousarray(out.astype(np.float32))

